# revision 5
# baseline (speedup 1.0000x reference)
"""Raw-Bacc (no TileContext) CenterLoss kernel.

loss = mean_b ||x_b - centers[labels_b]||^2  (+ tiny clip-floor constant)

Per core (128 batch rows):
  SP queue:  labels[0:64] spray, then x rows 0:64   (ring-FIFO: labels first)
  ACT queue: labels[64:128] spray, then x rows 64:128, then ACT casts x->bf16
  Pool:      single indirect DMA gathers centers[labels] -> SBUF, cast to bf16
  DVE:       diff = c - x (bf16, 2x rate); sq = diff*diff with f32 row-accum
  PE:        ones-matmul reduces the 128 partition values to one scalar
  DVE:       PSUM -> SBUF copy;  SP: DMA scalar out
Host sums the 8 per-core partials (the all-reduce) and divides by B.
"""

import numpy as np

_BATCH = 1024
_FEAT = 512
_NCLASSES = 10000
_NCORES = 8
_ROWS = _BATCH // _NCORES  # 128
_P = 128
_H = _ROWS // 2  # 64

_state = {}


def _build_nc_raw(decoy=True):
    import concourse.bass as bass
    import concourse.mybir as mybir
    from concourse import bacc

    f32 = mybir.dt.float32
    bf16 = mybir.dt.bfloat16
    i32 = mybir.dt.int32
    nc = bacc.Bacc("TRN2", target_bir_lowering=False, debug=False)
    x_d = nc.dram_tensor("x", [_ROWS, _FEAT], f32, kind="ExternalInput").ap()
    labels_d = nc.dram_tensor("labels", [_ROWS, 1], i32, kind="ExternalInput").ap()
    centers_d = nc.dram_tensor(
        "centers", [_NCLASSES, _FEAT], f32, kind="ExternalInput"
    ).ap()
    out_d = nc.dram_tensor("out", [1, 1], f32, kind="ExternalOutput").ap()

    from contextlib import ExitStack

    with ExitStack() as _es:
        labels_t = _es.enter_context(nc.sbuf_tensor("labels_t", [_ROWS, 1], i32))
        decoy_t = _es.enter_context(nc.sbuf_tensor("decoy_t", [1, 4], f32))
        x_t = _es.enter_context(nc.sbuf_tensor("x_t", [_P, _FEAT], f32))
        xb_t = _es.enter_context(nc.sbuf_tensor("xb_t", [_P, _FEAT], bf16))
        cb_t = _es.enter_context(nc.sbuf_tensor("cb_t", [_P, _FEAT], bf16))
        diff_t = _es.enter_context(nc.sbuf_tensor("diff_t", [_P, _FEAT], bf16))
        sq_t = _es.enter_context(nc.sbuf_tensor("sq_t", [_P, _FEAT], bf16))
        s1_t = _es.enter_context(nc.sbuf_tensor("s1_t", [_P, 1], f32))
        ones_t = _es.enter_context(nc.sbuf_tensor("ones_t", [_P, 1], f32))
        res_t = _es.enter_context(nc.sbuf_tensor("res_t", [1, 1], f32))
        acc_t = _es.enter_context(nc.psum_tensor("acc_t", [1, 1], f32))
        lab_sem = _es.enter_context(nc.semaphore("lab_sem"))
        lab_b_sem = _es.enter_context(nc.semaphore("lab_b_sem"))
        decoy_sem = _es.enter_context(nc.semaphore("decoy_sem"))
        x_sem = _es.enter_context(nc.semaphore("x_sem"))
        a_sem = _es.enter_context(nc.semaphore("a_sem"))
        c_sem = _es.enter_context(nc.semaphore("c_sem"))
        dve_sem = _es.enter_context(nc.semaphore("dve_sem"))
        m_sem = _es.enter_context(nc.semaphore("m_sem"))
        o_sem = _es.enter_context(nc.semaphore("o_sem"))
        # labels sprays lead each HWDGE ring; x row-halves queue behind them
        # on the same rings so the label descriptors drain first
        nc.sync.dma_start(labels_t.ap()[0:_H, :], labels_d[0:_H, :]).then_inc(
            lab_sem, 16
        )
        nc.scalar.dma_start(labels_t.ap()[_H:_P, :], labels_d[_H:_ROWS, :]).then_inc(
            lab_b_sem, 16
        )
        nc.sync.dma_start(x_t.ap()[0:_H, :], x_d[0:_H, :]).then_inc(x_sem, 16)
        nc.scalar.dma_start(x_t.ap()[_H:_P, :], x_d[_H:_ROWS, :]).then_inc(x_sem, 16)
        nc.vector.memset(ones_t.ap(), 1.0)

        if decoy:
            # tiny SWDGE DMA warms the Pool dynamic-DMA path so the real
            # gather's ucode drain is cheap
            nc.gpsimd.dma_start(decoy_t.ap(), centers_d[0:1, 0:4]).then_inc(
                decoy_sem, 16
            )

        nc.gpsimd.wait_ge(lab_sem, 16)
        nc.gpsimd.wait_ge(lab_b_sem, 16)
        # cast-on-gather: f32 center rows land in SBUF as bf16 (half the
        # drain bytes, enables 2x-rate DVE math)
        nc.gpsimd.indirect_dma_start(
            out=cb_t.ap(),
            out_offset=None,
            in_=centers_d,
            in_offset=bass.IndirectOffsetOnAxis(ap=labels_t.ap()[:, :1], axis=0),
        ).then_inc(c_sem, 16)
        if decoy:
            nc.gpsimd.wait_ge(decoy_sem, 16)

        # ACT casts x to bf16 while the gather is in flight
        nc.scalar.wait_ge(x_sem, 32)
        nc.scalar.copy(xb_t.ap(), x_t.ap()).then_inc(a_sem, 1)

        # post-gather: diff = c - x; s1 = rowsum(diff*diff)
        nc.vector.wait_ge(c_sem, 16)
        nc.vector.wait_ge(a_sem, 1)
        nc.vector.tensor_tensor(
            out=diff_t.ap(), in0=cb_t.ap(), in1=xb_t.ap(),
            op=mybir.AluOpType.subtract,
        ).then_inc(dve_sem, 1)
        nc.vector.wait_ge(dve_sem, 1)
        nc.vector.scalar_tensor_tensor(
            out=sq_t.ap(), in0=diff_t.ap(), scalar=1.0, in1=diff_t.ap(),
            op0=mybir.AluOpType.mult, op1=mybir.AluOpType.mult,
            accum_out=s1_t.ap(),
        ).then_inc(dve_sem, 1)

        nc.tensor.wait_ge(dve_sem, 2)
        nc.tensor.matmul(
            acc_t.ap(), lhsT=s1_t.ap(), rhs=ones_t.ap(), start=True, stop=True
        ).then_inc(m_sem, 1)

        nc.vector.wait_ge(m_sem, 1)
        nc.vector.tensor_copy(out=res_t.ap(), in_=acc_t.ap()).then_inc(dve_sem, 1)

        nc.sync.wait_ge(dve_sem, 3)
        nc.sync.dma_start(out_d, res_t.ap()).then_inc(o_sem, 16)

    nc.compile()
    return nc


def _run(x, labels, centers, trace=False, decoy=True):
    from concourse.bass_utils import run_bass_kernel_spmd

    key = ("nc", decoy)
    if key not in _state:
        _state[key] = _build_nc_raw(decoy=decoy)
    nc = _state[key]

    x = np.ascontiguousarray(np.asarray(x, dtype=np.float32)).reshape(
        _NCORES, _ROWS, _FEAT
    )
    lab = (
        np.ascontiguousarray(np.asarray(labels))
        .astype(np.int32)
        .reshape(_NCORES, _ROWS, 1)
    )
    cen = np.ascontiguousarray(np.asarray(centers, dtype=np.float32))
    in_maps = [{"x": x[i], "labels": lab[i], "centers": cen} for i in range(_NCORES)]
    res = run_bass_kernel_spmd(nc, in_maps, core_ids=list(range(_NCORES)), trace=trace)
    total = 0.0
    for r in res.results:
        total += float(r["out"][0, 0])
    loss = total / _BATCH + (_NCLASSES - 1) * 1e-12
    return np.float32(loss), res


def kernel(x, labels, centers):
    loss, _ = _run(x, labels, centers, trace=False, decoy=True)
    return loss
